# revision 7
# baseline (speedup 1.0000x reference)
"""Trainium2 Bass kernel for nn_ExpertAttention (MoE-routed LoRA attention).

Math (per batch element b):
  routing: argmax over softmax(enc(mean_t x) @ sw) -> expert r_b  [host, exact]
  output  = expert_mha_{r_b}(x_b) + common_mha(x_b)     (fwd scale == 1.0)
LoRA is folded host-side into dense effective weights (exact when LoRA B == 0,
which is how setup_inputs initializes it). Scores scaling (1/8) is folded into
Wq/bq host-side (exact: power of two). V-bias and output-bias fold into a
single final bias row added once (bv @ Wo + bo), so the device kernel is two
plain 12-head MHAs per element, sharing one output accumulator.

Device layout: everything contracts over features, so activations live
transposed ([feature, token]) except V (token-major, with a ones column per
head so each AV matmul also emits the softmax denominator row). Softmax skips
max-subtraction (|scores| is a few units at most: inputs are sane-scaled and
exp is fp32-safe up to 88) and normalization is applied per-head at the output
projection, where tokens sit on partitions and 1/denom is a per-partition
scalar. Matmuls run as float32r (FP22 multiply, fp32 accumulate): full PE
rate at free-dim >= 256, ~1.5e-4 relative error.

Data-parallel over batch: 8 elements per NeuronCore, 8 cores.
"""

import numpy as np

import concourse.bass as bass
import concourse.mybir as mybir
import concourse.tile as tile
from concourse.bass_utils import run_bass_kernel_spmd
from concourse.masks import make_identity
from concourse.vector_clock import ScopedClock

# ---------------------------------------------------------------------------
# Workaround: this walrus build rejects instructions carrying more than one
# sync-wait condition, but Tile freely assigns several. Split extra waits onto
# same-engine nops emitted immediately before the instruction (same-engine
# program order makes this equivalent).
# ---------------------------------------------------------------------------
_orig_add_instruction = tile.TileContext._add_instruction


def _split_waits_add_instruction(self, inst):
    si = inst.sync_info
    if si is not None and si.on_wait and len(si.on_wait) > 1 and inst.engine is not None:
        waits = list(si.on_wait)
        for w in waits[:-1]:
            nop = mybir.InstNoOp(
                name=self.nc.get_next_instruction_name(),
                ins=[],
                outs=[],
                engine=inst.engine,
            )
            nop.sync_info = mybir.SyncInfo(on_wait=[w], on_update=[])
            _orig_add_instruction(self, nop)
        si.on_wait = waits[-1:]
    _orig_add_instruction(self, inst)


def _patched_drain_and_barrier(self, tick_clock, wait_clock):
    collector = self.nc.sync.nop()
    wait_clock.add_sem_waits(collector.ins, ScopedClock({None: tick_clock.global_clock}))
    si = collector.ins.sync_info
    waits = list(si.on_wait) if si is not None else []
    if len(waits) > 1:
        si.on_wait = waits[:1]
        for w in waits[1:]:
            n = self.nc.sync.nop()
            n.ins.sync_info = mybir.SyncInfo(on_wait=[w], on_update=[])
    self.nc.sync.drain()
    self.nc.all_engine_barrier()
    assert self.sems is not None
    popped = self.nc._tile_sem_poison_stack.pop()
    assert popped is self._sem_poison
    self.nc.clear_and_free_semaphores(list(self.sems.allocated().values()))
    self.nc.all_engine_barrier()


tile.TileContext._add_instruction = _split_waits_add_instruction
tile.TileContext._drain_and_barrier = _patched_drain_and_barrier

# ---------------------------------------------------------------------------

F32 = mybir.dt.float32
F32R = mybir.dt.float32r
EXP = mybir.ActivationFunctionType.Exp
IDENT = mybir.ActivationFunctionType.Identity
COPY = mybir.ActivationFunctionType.Copy

B, S, D = 64, 512, 768
H, DH = 12, 64
NEG = -10000.0
NCORES = 8
BPC = B // NCORES  # batch elements per core
DC = D // 128      # feature chunks (6)
QC = S // 128      # query-token chunks (4)
KC = S // 128      # key-token chunks (4)
PAIRS = H // 2     # head pairs (6)
VW = DH + 1        # v columns per head incl. ones column (65)

_NC_CACHE = []


def _build_nc():
    nc = bass.Bass()
    x_d = nc.dram_tensor("x", [BPC, S, D], F32, kind="ExternalInput")
    wc_d = nc.dram_tensor("wc", [4, D, D], F32, kind="ExternalInput")
    we_d = nc.dram_tensor("we", [BPC, 4, D, D], F32, kind="ExternalInput")
    bqkc_d = nc.dram_tensor("bqkc", [128, 12], F32, kind="ExternalInput")
    bqke_d = nc.dram_tensor("bqke", [BPC, 128, 12], F32, kind="ExternalInput")
    maskb_d = nc.dram_tensor("maskb", [BPC, 128, KC], F32, kind="ExternalInput")
    bfin_d = nc.dram_tensor("bfin", [BPC, 128, D], F32, kind="ExternalInput")
    ones_d = nc.dram_tensor("ones", [128, H], F32, kind="ExternalInput")
    out_d = nc.dram_tensor("out", [BPC, S, D], F32, kind="ExternalOutput")

    with tile.TileContext(nc) as tc:
        with (
            tc.tile_pool(name="const", bufs=1) as constp,
            tc.tile_pool(name="wgt", bufs=3) as wgtp,
            tc.tile_pool(name="xn", bufs=1) as xnp,
            tc.tile_pool(name="xt", bufs=1) as xtp,
            tc.tile_pool(name="qkv", bufs=1) as qkvp,
            tc.tile_pool(name="exp", bufs=4) as expp,
            tc.tile_pool(name="ctx", bufs=7) as ctxp,
            tc.tile_pool(name="dn", bufs=3) as dnp,
            tc.tile_pool(name="acc", bufs=1) as accp,
            tc.tile_pool(name="tmp", bufs=4) as tmpp,
            tc.tile_pool(name="small", bufs=2) as smallp,
        ):
            ident = constp.tile([128, 128], F32)
            make_identity(nc, ident[:])
            onest = constp.tile([128, H], F32R)
            nc.sync.dma_start(onest[:], ones_d.ap().bitcast(F32R))

            for b in range(BPC):
                # ---- per-element small inputs ----
                mbt = smallp.tile([128, KC], F32, tag="mbt")
                nc.sync.dma_start(mbt[:], maskb_d.ap()[b])
                bfint = smallp.tile([128, D], F32, tag="bfint")
                nc.sync.dma_start(bfint[:], bfin_d.ap()[b])
                bqket = smallp.tile([128, 12], F32, tag="bqket")
                nc.sync.dma_start(bqket[:], bqke_d.ap()[b])
                bqkct = smallp.tile([128, 12], F32, tag="bqkct")
                nc.sync.dma_start(bqkct[:], bqkc_d.ap())

                # ---- load x, build x^T ----
                xnt = xnp.tile([128, QC * D], F32)
                nc.sync.dma_start(
                    xnt[:].rearrange("p (t f) -> p t f", t=QC),
                    x_d.ap()[b].rearrange("(t p) f -> p t f", p=128),
                )
                xT = xtp.tile([128, DC * S], F32R)
                with tc.tile_pool(name="psT", bufs=4, space="PSUM") as psT:
                    for dc in range(DC):
                        for tc4 in range(QC):
                            pt = psT.tile([128, 128], F32)
                            nc.tensor.transpose(
                                pt[:], xnt[:, D * tc4 + 128 * dc : D * tc4 + 128 * dc + 128],
                                ident[:],
                            )
                            nc.vector.tensor_copy(
                                xT[:, S * dc + 128 * tc4 : S * dc + 128 * tc4 + 128], pt[:]
                            )

                acc = accp.tile([128, QC * D], F32)

                for mha in range(2):  # 0 = common, 1 = expert
                    if mha == 0:
                        w_src = [wc_d.ap()[i] for i in range(4)]
                        bqk = bqkct
                    else:
                        w_src = [we_d.ap()[b, i] for i in range(4)]
                        bqk = bqket

                    def load_w(i):
                        t = wgtp.tile([128, DC * D], F32R, tag="w")
                        nc.sync.dma_start(
                            t[:].rearrange("p (c f) -> p c f", c=DC),
                            w_src[i].rearrange("(c p) f -> p c f", p=128).bitcast(F32R),
                        )
                        return t

                    # ---- q^T, k^T projections (transposed, biased) ----
                    qT = qkvp.tile([128, DC * S], F32R, tag="qT")
                    kT = qkvp.tile([128, DC * S], F32R, tag="kT")
                    with tc.tile_pool(name="psP", bufs=2, space="PSUM") as psP:
                        for dst, wi, bcol in ((qT, 0, 0), (kT, 1, 6)):
                            wt = load_w(wi)
                            for mc in range(DC):
                                ps = psP.tile([128, S], F32)
                                for kc in range(DC):
                                    nc.tensor.matmul(
                                        ps[:],
                                        wt[:, D * kc + 128 * mc : D * kc + 128 * mc + 128],
                                        xT[:, S * kc : S * kc + S],
                                        start=(kc == 0),
                                        stop=(kc == DC - 1),
                                    )
                                nc.scalar.activation(
                                    dst[:, S * mc : S * mc + S], ps[:], IDENT,
                                    bias=bqk[:, bcol + mc : bcol + mc + 1],
                                )

                    # ---- v (token-major, with ones columns) ----
                    vaug = qkvp.tile([128, QC * H * VW], F32R, tag="vaug")
                    with tc.tile_pool(name="psV", bufs=2, space="PSUM") as psV:
                        wt = load_w(2)
                        for tc4 in range(QC):
                            psv = psV.tile([128, D], F32)
                            for kc in range(DC):
                                for n0, nw in ((0, 512), (512, 256)):
                                    nc.tensor.matmul(
                                        psv[:, n0 : n0 + nw],
                                        xT[:, S * kc + 128 * tc4 : S * kc + 128 * tc4 + 128],
                                        wt[:, D * kc + n0 : D * kc + n0 + nw],
                                        start=(kc == 0),
                                        stop=(kc == DC - 1),
                                    )
                            vblk = vaug[:, H * VW * tc4 : H * VW * (tc4 + 1)].rearrange(
                                "p (h f) -> p h f", h=H
                            )
                            nc.vector.tensor_copy(
                                vblk[:, :, 0:DH],
                                psv[:].rearrange("p (h f) -> p h f", h=H),
                            )
                            nc.vector.tensor_copy(
                                vblk[:, :, DH : DH + 1],
                                onest[:].rearrange("p (h o) -> p h o", o=1),
                            )

                    # ---- attention (per head pair, row-packed) ----
                    wt_o = load_w(3)
                    ctxts = []
                    dfps = []
                    with (
                        tc.tile_pool(name="psS", bufs=2, space="PSUM") as psS,
                        tc.tile_pool(name="psA", bufs=2, space="PSUM") as psA,
                    ):
                        for p in range(PAIRS):
                            av0 = psA.tile([VW, S], F32, tag="av0")
                            av1 = psA.tile([VW, S], F32, tag="av1")
                            for kc in range(KC):
                                s0 = psS.tile([128, S], F32, tag="s0")
                                s1 = psS.tile([128, S], F32, tag="s1")
                                nc.tensor.matmul(
                                    s0[:],
                                    kT[0:64, S * p + 128 * kc : S * p + 128 * kc + 128],
                                    qT[0:64, S * p : S * p + S],
                                    start=True, stop=True,
                                )
                                nc.tensor.matmul(
                                    s1[:],
                                    kT[64:128, S * p + 128 * kc : S * p + 128 * kc + 128],
                                    qT[64:128, S * p : S * p + S],
                                    start=True, stop=True,
                                )
                                e0 = expp.tile([128, S], F32R, tag="e0")
                                e1 = expp.tile([128, S], F32R, tag="e1")
                                nc.scalar.activation(
                                    e0[:], s0[:], EXP, bias=mbt[:, kc : kc + 1]
                                )
                                nc.scalar.activation(
                                    e1[:], s1[:], EXP, bias=mbt[:, kc : kc + 1]
                                )
                                base = H * VW * kc
                                nc.tensor.matmul(
                                    av0[:],
                                    vaug[:, base + VW * 2 * p : base + VW * 2 * p + VW],
                                    e0[:],
                                    start=(kc == 0), stop=(kc == KC - 1),
                                )
                                nc.tensor.matmul(
                                    av1[:],
                                    vaug[:, base + VW * (2 * p + 1) : base + VW * (2 * p + 1) + VW],
                                    e1[:],
                                    start=(kc == 0), stop=(kc == KC - 1),
                                )
                            ctxt = ctxp.tile([128, S], F32R, tag="ctxt")
                            nc.vector.tensor_copy(ctxt[0:64, :], av0[0:64, :])
                            nc.vector.tensor_copy(ctxt[64:128, :], av1[0:64, :])
                            dfp = dnp.tile([1, 2 * S], F32, tag="dflat")
                            nc.scalar.copy(dfp[0:1, 0:S], av0[64:65, :])
                            nc.scalar.copy(dfp[0:1, S : 2 * S], av1[64:65, :])
                            ctxts.append(ctxt)
                            dfps.append(dfp)

                    # ---- denominators -> per-token reciprocal scalars ----
                    dpart = dnp.tile([H, S], F32, tag="dpart")
                    for p in range(PAIRS):
                        for j in range(2):
                            nc.sync.dma_start(
                                dpart[2 * p + j : 2 * p + j + 1, :],
                                dfps[p][0:1, S * j : S * j + S],
                            )
                    recip = dnp.tile([128, H * QC], F32, tag="recip")
                    with tc.tile_pool(name="psD", bufs=2, space="PSUM") as psD:
                        for qc in range(QC):
                            pd = psD.tile([128, H], F32)
                            nc.tensor.transpose(
                                pd[:], dpart[0:H, 128 * qc : 128 * qc + 128],
                                ident[0:H, 0:H],
                            )
                            nc.vector.reciprocal(recip[:, H * qc : H * qc + H], pd[:])

                    # ---- output projection, normalize, accumulate ----
                    with tc.tile_pool(name="psO", bufs=2, space="PSUM") as psO:
                        for qc in range(QC):
                            av = acc[:, D * qc : D * qc + D]
                            for p in range(PAIRS):
                                po0 = psO.tile([128, D], F32, tag="po0")
                                po1 = psO.tile([128, D], F32, tag="po1")
                                for n0, nw in ((0, 512), (512, 256)):
                                    nc.tensor.matmul(
                                        po0[:, n0 : n0 + nw],
                                        ctxts[p][0:64, 128 * qc : 128 * qc + 128],
                                        wt_o[0:64, D * p + n0 : D * p + n0 + nw],
                                        start=True, stop=True,
                                    )
                                    nc.tensor.matmul(
                                        po1[:, n0 : n0 + nw],
                                        ctxts[p][64:128, 128 * qc : 128 * qc + 128],
                                        wt_o[64:128, D * p + n0 : D * p + n0 + nw],
                                        start=True, stop=True,
                                    )
                                t0 = tmpp.tile([128, D], F32, tag="t0")
                                nc.scalar.activation(
                                    t0[:], po0[:], COPY,
                                    scale=recip[:, H * qc + 2 * p : H * qc + 2 * p + 1],
                                )
                                if mha == 0 and p == 0:
                                    nc.vector.tensor_add(av[:], bfint[:], t0[:])
                                else:
                                    nc.vector.tensor_add(av[:], av[:], t0[:])
                                t1 = tmpp.tile([128, D], F32, tag="t1")
                                nc.vector.tensor_scalar_mul(
                                    t1[:], po1[:],
                                    recip[:, H * qc + 2 * p + 1 : H * qc + 2 * p + 2],
                                )
                                nc.vector.tensor_add(av[:], av[:], t1[:])

                # ---- store ----
                nc.sync.dma_start(
                    out_d.ap()[b].rearrange("(t p) f -> p t f", p=128),
                    acc[:].rearrange("p (t f) -> p t f", t=QC),
                )

    return nc


def _host_prep(hidden_states, attention_mask, params):
    x = np.ascontiguousarray(np.asarray(hidden_states, dtype=np.float32))
    mask = np.asarray(attention_mask, dtype=np.float32)
    p_enc_W = np.asarray(params["enc_W"], dtype=np.float32)
    p_enc_b = np.asarray(params["enc_b"], dtype=np.float32)
    p_sw_W = np.asarray(params["sw_W"], dtype=np.float32)
    p_sw_b = np.asarray(params["sw_b"], dtype=np.float32)

    # routing (argmax of softmax == argmax of logits), float64 host math
    pooled = x.astype(np.float64).mean(axis=1)
    h_enc = pooled @ p_enc_W.astype(np.float64) + p_enc_b.astype(np.float64)
    logits = h_enc @ p_sw_W.astype(np.float64) + p_sw_b.astype(np.float64)
    routes = np.argmax(logits, axis=-1).astype(np.int64)

    def getw(d, nm):
        return np.asarray(d[nm], dtype=np.float32)

    def eff(d, nm):
        W = getw(d, "W" + nm)
        if "A" + nm in d:
            A = getw(d, "A" + nm)
            Bm = getw(d, "B" + nm)
            if np.any(Bm):
                W = W + A @ Bm
        return W

    com = params["common"]
    Wq_c = eff(com, "q") * 0.125
    Wk_c = eff(com, "k")
    Wv_c = eff(com, "v")
    Wo_c = eff(com, "o")
    wc = np.ascontiguousarray(np.stack([Wq_c, Wk_c, Wv_c, Wo_c]))

    def fold_b(bvec):  # [768] -> [128, 6] column-chunk layout
        return np.ascontiguousarray(bvec.reshape(6, 128).T)

    bqkc = np.ascontiguousarray(
        np.concatenate(
            [fold_b(getw(com, "bq") * 0.125), fold_b(getw(com, "bk"))], axis=1
        )
    )

    wq_e, wk_e, wv_e, wo_e, bqk_e, bfin_e = [], [], [], [], [], []
    for e in range(len(params["experts"])):
        ex = params["experts"][e]
        wq_e.append(eff(ex, "q") * 0.125)
        wk_e.append(eff(ex, "k"))
        wv_e.append(eff(ex, "v"))
        wo_e.append(eff(ex, "o"))
        bqk_e.append(
            np.concatenate(
                [fold_b(getw(ex, "bq") * 0.125), fold_b(getw(ex, "bk"))], axis=1
            )
        )
        bfin_e.append(
            getw(com, "bo") + getw(com, "bv") @ Wo_c
            + getw(ex, "bo") + getw(ex, "bv") @ wo_e[e]
        )

    maskb = ((1.0 - mask) * NEG).reshape(B, KC, 128).transpose(0, 2, 1)
    maskb = np.ascontiguousarray(maskb.astype(np.float32))

    in_maps = []
    for c in range(NCORES):
        sl = slice(c * BPC, (c + 1) * BPC)
        rts = routes[sl]
        in_maps.append(
            {
                "x": x[sl],
                "wc": wc,
                "we": np.ascontiguousarray(
                    np.stack(
                        [
                            np.stack([wq_e[r], wk_e[r], wv_e[r], wo_e[r]])
                            for r in rts
                        ]
                    )
                ),
                "bqkc": bqkc,
                "bqke": np.ascontiguousarray(np.stack([bqk_e[r] for r in rts])),
                "maskb": maskb[sl],
                "ones": np.ones((128, H), np.float32),
                "bfin": np.ascontiguousarray(
                    np.broadcast_to(
                        np.stack([bfin_e[r] for r in rts])[:, None, :],
                        (BPC, 128, D),
                    ).copy()
                ),
            }
        )
    return in_maps


def kernel(hidden_states, attention_mask, params):
    in_maps = _host_prep(hidden_states, attention_mask, params)
    if not _NC_CACHE:
        _NC_CACHE.append(_build_nc())
    nc = _NC_CACHE[0]
    res = run_bass_kernel_spmd(nc, in_maps, core_ids=list(range(NCORES)))
    out = np.concatenate([res.results[c]["out"] for c in range(NCORES)], axis=0)
    return np.ascontiguousarray(out.astype(np.float32))
